# revision 15
# baseline (speedup 1.0000x reference)
"""BERT input representation kernel for 8 TRN2 NeuronCores.

Math (reference):
    x1  = x @ W_emb + b_emb                      # [B,S,D]
    seg = einsum('bnsd,s->bnd', x1.reshape(B,S/8,8,D), w_seg) + b_seg
    out = (x1.reshape(...) + seg[:,:,None,:]).reshape(B,S,D) + PE(S,D)

Folded form used here (exact algebra):
    out[b,s,:] = (A @ x[b])[s,:] @ W_emb + bias[s,:]
where A = I + blockdiag(ones(8,1) @ w_seg[None,:]) mixes rows within each
8-row segment, and bias[s,:] = PE[s,:] + b_emb*(1 + sum(w_seg)) + b_seg.

Sharding: pure data-parallel over batch; each of 8 cores handles 8
batches (4096 rows = 32 row-tiles of 128 rows = 16 tile-pair groups).

v3 schedule:
  - output stored bf16 (host upcasts to f32): store traffic halves
  - prologue: all of x loads in 3 DMAs; all 16 transpose+segment-mix
    matmuls run into one 4-bank PSUM workspace; 4 big ACT copies build
    the resident bf16 x~^T.  The steady loop then has no PE<->ACT
    ping-pong.
  - per pair j: one [128,2048] f32 PSUM tile (4 banks, 2 bufs = all 8),
    4 mains (start=True, FD=512).  Epilogue split: DVE fused
    drain+bias tensor_tensor on cols [0:XV) (PSUM 1x mode), ACT
    plain-drains [XV:2048) (1x), DVE then adds bias there as a bf16
    SBUF tensor_tensor (2x packed mode).  The DVE add for pair j is
    emitted after pair j+1's fused op (software pipelining) so DVE
    never idles waiting for ACT.
  - two 256 KiB bf16 stores per pair on the sync HWDGE ring
"""

import sys

if "/opt/trn_rl_repo" not in sys.path:
    sys.path.insert(0, "/opt/trn_rl_repo")

import ml_dtypes
import numpy as np

import concourse.bacc as bacc
import concourse.mybir as mybir
import concourse.tile as tile
from concourse.bass_utils import run_bass_kernel_spmd

B, S, F, D, SEG = 64, 512, 64, 1024, 8
N_CORES = 8
B_LOC = B // N_CORES          # batches per core
ROWS = B_LOC * S              # 4096 rows per core
TILE_P = 128                  # rows per tile
N_TILES = ROWS // TILE_P      # 32
N_PAIR = N_TILES // 2         # 16 tile-pairs
N_BIAS = S // TILE_P          # 4 distinct bias row-tiles
PW = 2 * D                    # 2048 cols per pair psum tile
XV = 512                      # DVE fused drain+bias covers cols [0:XV)

_NC_CACHE = None
DEFAULT_CFG = {"XV": XV, "INJECT": False, "LATE23": True, "XSPLIT": False,
               "WIDE0": False, "TAILSPLIT": False}


def _build_nc(cfg=None):
    cfg = dict(DEFAULT_CFG, **(cfg or {}))
    xv, inject = cfg["XV"], cfg["INJECT"]
    late23, xsplit = cfg["LATE23"], cfg["XSPLIT"]
    wide0, tailsplit = cfg["WIDE0"], cfg["TAILSPLIT"]
    nc = bacc.Bacc("TRN2", target_bir_lowering=False, debug=False,
                   num_devices=N_CORES)
    # x pre-rearranged on host (layout + cast to bf16):
    # xr[p, i*F:(i+1)*F] = x[i*128+p]; cols [0:128] = A^T
    x_d = nc.declare_dram_parameter("x", [TILE_P, TILE_P + N_TILES * F],
                                    mybir.dt.bfloat16, isOutput=False)
    # combined constants [128, 5120]: cols [0:1024]=W stacked twice
    # (partitions 0-63 and 64-127 both hold W) | [1024:5120]=bias0..3
    cc_d = nc.declare_dram_parameter("cc", [TILE_P, 5 * D + TILE_P],
                                     mybir.dt.bfloat16, isOutput=False)
    out_d = nc.declare_dram_parameter("out", [ROWS, D], mybir.dt.bfloat16,
                                      isOutput=True)

    with tile.TileContext(nc) as tc:
        with (
            tc.tile_pool(name="const", bufs=1) as cpool,
            tc.tile_pool(name="outp", bufs=4) as opool,
            tc.tile_pool(name="ps", bufs=2, space="PSUM") as psp,
        ):
            # loads: sync ring carries A^T+x then the stores; scalar ring
            # carries W and the bias tiles in need-order.
            n0 = 5 if wide0 else 2     # cols of x_d in the first DMA
            at_x0 = cpool.tile([TILE_P, n0 * TILE_P], mybir.dt.bfloat16)
            nc.sync.dma_start(at_x0[:], x_d[:, 0:n0 * TILE_P])
            at_ap = at_x0[:, 0:TILE_P]
            CCW = 5 * D + TILE_P
            BB = D + TILE_P            # bias block base in cc
            cc_sb = cpool.tile([TILE_P, CCW], mybir.dt.bfloat16)
            nc.scalar.dma_start(cc_sb[:, 0:BB], cc_d[:, 0:BB])
            xr_sb = cpool.tile([TILE_P, (N_PAIR - 1) * TILE_P],
                               mybir.dt.bfloat16)
            if wide0:
                nc.sync.dma_start(xr_sb[:, 3 * TILE_P:7 * TILE_P],
                                  x_d[:, 5 * TILE_P:9 * TILE_P])
            elif xsplit:
                nc.sync.dma_start(xr_sb[:, 0:3 * TILE_P],
                                  x_d[:, 2 * TILE_P:5 * TILE_P])
                nc.sync.dma_start(xr_sb[:, 3 * TILE_P:7 * TILE_P],
                                  x_d[:, 5 * TILE_P:9 * TILE_P])
            else:
                nc.sync.dma_start(xr_sb[:, 0:7 * TILE_P],
                                  x_d[:, 2 * TILE_P:9 * TILE_P])
            nc.sync.dma_start(xr_sb[:, 7 * TILE_P:15 * TILE_P],
                              x_d[:, 9 * TILE_P:17 * TILE_P])
            nc.scalar.dma_start(cc_sb[:, BB:BB + 2 * D],
                                cc_d[:, BB:BB + 2 * D])
            nc.scalar.dma_start(cc_sb[:, BB + 2 * D:CCW],
                                cc_d[:, BB + 2 * D:CCW])
            i_ap = cc_sb[:, D:D + TILE_P]

            def bias_pair(j):
                base = BB + ((2 * j) % N_BIAS) * D
                return cc_sb[:, base:base + PW]

            def w_ap(u, lo, hi):
                return cc_sb[64 * u:64 * u + F, lo:hi]

            # resident x~^T (bf16): xt_sb[64u+f, 128j+n] = x~[2j+u, n, f]
            xt_sb = cpool.tile([TILE_P, N_PAIR * TILE_P], mybir.dt.bfloat16)

            # prologue: the 16 transpose+mix matmuls use banks 0-1 of
            # pair 0's and pair 1's psum tiles as workspace (so each
            # pair's m3/m4 banks carry no WAR dependency on the late
            # transpose blocks), interleaved with the xt_sb copies.
            pair0 = psp.tile([TILE_P, PW], mybir.dt.float32,
                             name="pair", tag="pair")
            pair1 = psp.tile([TILE_P, PW], mybir.dt.float32,
                             name="pair", tag="pair")

            def tr_block(b4):
                # 4 transpose+mix matmuls into workspace banks, then one
                # bulk copy to xt_sb (ACT for even blocks, DVE for odd)
                wtile = pair0 if b4 < 2 else pair1
                wcol = 512 * (b4 % 2)
                for k in range(4):
                    pr = 4 * b4 + k
                    if wide0 and pr < 4:
                        src = at_x0[:, TILE_P * (pr + 1):TILE_P * (pr + 2)]
                    elif pr == 0:
                        src = at_x0[:, TILE_P:2 * TILE_P]
                    else:
                        src = xr_sb[:, 128 * (pr - 1):128 * pr]
                    nc.tensor.matmul(wtile[:, wcol + 128 * k:
                                           wcol + 128 * (k + 1)],
                                     src, at_ap, start=True, stop=True)
                if b4 % 2 == 0:
                    nc.scalar.copy(xt_sb[:, 512 * b4:512 * (b4 + 1)],
                                   wtile[:, wcol:wcol + 512])
                else:
                    nc.vector.tensor_copy(xt_sb[:, 512 * b4:512 * (b4 + 1)],
                                          wtile[:, wcol:wcol + 512])

            state = {"prev": None}

            def pair_body(j):
                bias = bias_pair(j)
                pair = (pair0 if j == 0 else pair1 if j == 1 else psp.tile(
                    [TILE_P, PW], mybir.dt.float32, name="pair", tag="pair"))
                lhs0 = xt_sb[0:64, 128 * j:128 * (j + 1)]
                lhs1 = xt_sb[64:128, 128 * j:128 * (j + 1)]
                nc.tensor.matmul(pair[:, 0:512], lhs0,
                                 w_ap(0, 0, 512), start=True, stop=True)
                nc.tensor.matmul(pair[:, 512:1024], lhs0,
                                 w_ap(0, 512, 1024), start=True, stop=True)
                nc.tensor.matmul(pair[:, 1024:1536], lhs1,
                                 w_ap(1, 0, 512), start=True, stop=True)
                nc.tensor.matmul(pair[:, 1536:2048], lhs1,
                                 w_ap(1, 512, 1024), start=True, stop=True)
                if inject:
                    nc.tensor.matmul(pair[:, 1536:2048], i_ap,
                                     bias[:, 1536:2048], start=False,
                                     stop=True, skip_group_check=True)
                o_t = opool.tile([TILE_P, PW], mybir.dt.bfloat16,
                                 name="o_t")
                # DVE fused drain+bias (PSUM 1x), ACT plain drain (1x)
                nc.vector.tensor_add(o_t[:, 0:xv], pair[:, 0:xv],
                                     bias[:, 0:xv])
                nc.scalar.copy(o_t[:, xv:PW], pair[:, xv:PW])
                if state["prev"] is not None:
                    _finish_pair(nc, out_d, *state["prev"], cfg)
                state["prev"] = (j, o_t, bias)

            # ramp: pair 0 runs between transpose blocks so its mains are
            # not queued behind the full transpose sweep on PE/ACT
            tr_block(0)
            tr_block(1)
            pair_body(0)
            if late23:
                with tc.tile_wait_until(0.004):
                    tr_block(2)
                    tr_block(3)
            else:
                tr_block(2)
                tr_block(3)
            for j in range(1, N_PAIR):
                pair_body(j)
            _finish_pair(nc, out_d, *state["prev"], cfg)
    nc.compile()
    return nc


def _finish_pair(nc, out_d, j, o_t, bias, cfg):
    # bias add for ACT's drained region: bf16 SBUF tensor_tensor (2x),
    # in place; then both stores.
    xv = cfg["XV"]
    hi = 1536 if cfg["INJECT"] else PW
    nc.vector.tensor_add(o_t[:, xv:hi], o_t[:, xv:hi], bias[:, xv:hi])
    if cfg["TAILSPLIT"] and j == N_PAIR - 1:
        for q in range(4):
            r0 = 256 * j + 128 * (q // 2)
            c0 = 512 * (q % 2)
            nc.sync.dma_start(out_d[r0:r0 + 128, c0:c0 + 512],
                              o_t[:, 512 * q:512 * (q + 1)])
        return
    nc.sync.dma_start(out_d[256 * j:256 * j + 128, :], o_t[:, 0:D])
    nc.sync.dma_start(out_d[256 * j + 128:256 * j + 256, :], o_t[:, D:PW])


def _host_constants(W_emb, b_emb, w_seg, b_seg):
    # sinusoidal positional encoding, float32, same formula as the reference
    pos = np.arange(S, dtype=np.float32)[:, None]
    div = np.exp(np.arange(0, D, 2, dtype=np.float32)
                 * (-np.log(10000.0) / D)).astype(np.float32)
    ang = pos * div
    pe = np.zeros((S, D), np.float32)
    pe[:, 0::2] = np.sin(ang)
    pe[:, 1::2] = np.cos(ang)

    bias = (pe + b_emb[None, :] * (np.float32(1.0) + w_seg.sum())
            + b_seg[0]).astype(np.float32)
    # rearrange to [128, 4*D]: column block j holds bias rows j*128..j*128+127
    bias_r = np.ascontiguousarray(
        bias.reshape(N_BIAS, TILE_P, D).transpose(1, 0, 2).reshape(
            TILE_P, N_BIAS * D)).astype(ml_dtypes.bfloat16)

    blk = np.eye(SEG, dtype=np.float32) + w_seg[:, None] * np.ones(
        (1, SEG), np.float32)
    at = np.kron(np.eye(TILE_P // SEG, dtype=np.float32), blk).astype(
        ml_dtypes.bfloat16)

    wb = np.vstack([W_emb, W_emb]).astype(ml_dtypes.bfloat16)
    ident = np.eye(TILE_P, dtype=np.float32).astype(ml_dtypes.bfloat16)
    # combined consts: [W2|I128|bias0..3] as [128, 5*D+128] bf16
    cc = np.ascontiguousarray(np.concatenate([wb, ident, bias_r], axis=1))
    return at, cc


def _prepare_in_maps(x, W_emb, b_emb, w_seg, b_seg):
    x = np.ascontiguousarray(np.asarray(x, dtype=np.float32))
    W_emb = np.asarray(W_emb, dtype=np.float32)
    b_emb = np.asarray(b_emb, dtype=np.float32)
    w_seg = np.asarray(w_seg, dtype=np.float32)
    b_seg = np.asarray(b_seg, dtype=np.float32)

    at, cc = _host_constants(W_emb, b_emb, w_seg, b_seg)

    in_maps = []
    for c in range(N_CORES):
        xs = x[c * B_LOC:(c + 1) * B_LOC].reshape(ROWS, F)
        # rearrange [32 tiles, 128 rows, F] -> [128, 32*F], bf16 staging
        xr = np.ascontiguousarray(
            xs.reshape(N_TILES, TILE_P, F).transpose(1, 0, 2).reshape(
                TILE_P, N_TILES * F)).astype(ml_dtypes.bfloat16)
        in_maps.append(
            {"x": np.ascontiguousarray(np.concatenate([at, xr], axis=1)),
             "cc": cc})
    return in_maps


def kernel(x, W_emb, b_emb, w_seg, b_seg):
    in_maps = _prepare_in_maps(x, W_emb, b_emb, w_seg, b_seg)

    global _NC_CACHE
    if _NC_CACHE is None:
        _NC_CACHE = _build_nc()

    res = run_bass_kernel_spmd(_NC_CACHE, in_maps,
                               core_ids=list(range(N_CORES)))
    out = np.concatenate(
        [np.asarray(res.results[c]["out"]).astype(np.float32).reshape(
            B_LOC, S, D) for c in range(N_CORES)], axis=0)
    return out


# revision 17
# speedup vs baseline: 1.0066x; 1.0066x over previous
"""BERT input representation kernel for 8 TRN2 NeuronCores.

Math (reference):
    x1  = x @ W_emb + b_emb                      # [B,S,D]
    seg = einsum('bnsd,s->bnd', x1.reshape(B,S/8,8,D), w_seg) + b_seg
    out = (x1.reshape(...) + seg[:,:,None,:]).reshape(B,S,D) + PE(S,D)

Folded form used here (exact algebra):
    out[b,s,:] = (A @ x[b])[s,:] @ W_emb + bias[s,:]
where A = I + blockdiag(ones(8,1) @ w_seg[None,:]) mixes rows within each
8-row segment, and bias[s,:] = PE[s,:] + b_emb*(1 + sum(w_seg)) + b_seg.

Sharding: pure data-parallel over batch; each of 8 cores handles 8
batches (4096 rows = 32 row-tiles of 128 rows = 16 tile-pair groups).

Schedule (v6):
  - output stored bf16 (host upcasts to f32): store traffic halves vs
    f32, 8.4 MiB/core written, ~2 MiB read
  - prologue: x loads in 3 sync-ring DMAs, W+identity then bias tiles
    on the scalar ring; the 16 transpose+segment-mix matmuls use banks
    0-1 of pair 0/1's PSUM tiles as workspace, with bulk PSUM->SBUF
    copies (alternating ACT/DVE) building the resident bf16 x~^T.
    Pair 0's body is emitted between transpose blocks 1 and 2, and
    blocks 2-3 carry a late scheduling hint (tile_wait_until), so the
    first store launches as early as possible.
  - steady loop per pair j: one [128,2048] f32 PSUM tile (4 banks,
    2 bufs = all of PSUM), 4 mains (start=True, FD=512).  Epilogue is
    split across both PSUM-capable engines: DVE does a fused
    drain+bias tensor_tensor on cols [0:XV) (PSUM 1x mode), ACT
    plain-drains [XV:2048) (1x), and DVE adds bias there as a bf16
    SBUF tensor_tensor (2x packed mode).  The DVE add for pair j is
    emitted after pair j+1's fused op (software pipelining) so DVE
    never idles waiting for ACT.  Steady cadence ~1.7 us/pair,
    ACT/DVE-balanced (the f32 PSUM drain at 1 elem/cycle/lane is the
    hard floor on TRN2; matmul cannot write bf16 PSUM here).
  - two 256 KiB bf16 stores per pair on the sync HWDGE ring
"""

import sys

if "/opt/trn_rl_repo" not in sys.path:
    sys.path.insert(0, "/opt/trn_rl_repo")

import ml_dtypes
import numpy as np

import concourse.bacc as bacc
import concourse.mybir as mybir
import concourse.tile as tile
from concourse.bass_utils import run_bass_kernel_spmd

B, S, F, D, SEG = 64, 512, 64, 1024, 8
N_CORES = 8
B_LOC = B // N_CORES          # batches per core
ROWS = B_LOC * S              # 4096 rows per core
TILE_P = 128                  # rows per tile
N_TILES = ROWS // TILE_P      # 32
N_PAIR = N_TILES // 2         # 16 tile-pairs
N_BIAS = S // TILE_P          # 4 distinct bias row-tiles
PW = 2 * D                    # 2048 cols per pair psum tile
XV = 576                      # DVE fused drain+bias covers cols [0:XV)

_NC_CACHE = None
DEFAULT_CFG = {"XV": XV, "INJECT": False, "LATE23": True, "XSPLIT": False,
               "WIDE0": False, "TAILSPLIT": False, "OBUFS": 4}


def _build_nc(cfg=None):
    cfg = dict(DEFAULT_CFG, **(cfg or {}))
    xv, inject = cfg["XV"], cfg["INJECT"]
    late23, xsplit = cfg["LATE23"], cfg["XSPLIT"]
    wide0, tailsplit = cfg["WIDE0"], cfg["TAILSPLIT"]
    obufs = cfg["OBUFS"]
    nc = bacc.Bacc("TRN2", target_bir_lowering=False, debug=False,
                   num_devices=N_CORES)
    # x pre-rearranged on host (layout + cast to bf16):
    # xr[p, i*F:(i+1)*F] = x[i*128+p]; cols [0:128] = A^T
    x_d = nc.declare_dram_parameter("x", [TILE_P, TILE_P + N_TILES * F],
                                    mybir.dt.bfloat16, isOutput=False)
    # combined constants [128, 5120]: cols [0:1024]=W stacked twice
    # (partitions 0-63 and 64-127 both hold W) | [1024:5120]=bias0..3
    cc_d = nc.declare_dram_parameter("cc", [TILE_P, 5 * D + TILE_P],
                                     mybir.dt.bfloat16, isOutput=False)
    out_d = nc.declare_dram_parameter("out", [ROWS, D], mybir.dt.bfloat16,
                                      isOutput=True)

    with tile.TileContext(nc) as tc:
        with (
            tc.tile_pool(name="const", bufs=1) as cpool,
            tc.tile_pool(name="outp", bufs=obufs) as opool,
            tc.tile_pool(name="ps", bufs=2, space="PSUM") as psp,
        ):
            # loads: sync ring carries A^T+x then the stores; scalar ring
            # carries W and the bias tiles in need-order.
            n0 = 5 if wide0 else 2     # cols of x_d in the first DMA
            at_x0 = cpool.tile([TILE_P, n0 * TILE_P], mybir.dt.bfloat16)
            nc.sync.dma_start(at_x0[:], x_d[:, 0:n0 * TILE_P])
            at_ap = at_x0[:, 0:TILE_P]
            CCW = 5 * D + TILE_P
            BB = D + TILE_P            # bias block base in cc
            cc_sb = cpool.tile([TILE_P, CCW], mybir.dt.bfloat16)
            nc.scalar.dma_start(cc_sb[:, 0:BB], cc_d[:, 0:BB])
            xr_sb = cpool.tile([TILE_P, (N_PAIR - 1) * TILE_P],
                               mybir.dt.bfloat16)
            if wide0:
                nc.sync.dma_start(xr_sb[:, 3 * TILE_P:7 * TILE_P],
                                  x_d[:, 5 * TILE_P:9 * TILE_P])
            elif xsplit:
                nc.sync.dma_start(xr_sb[:, 0:3 * TILE_P],
                                  x_d[:, 2 * TILE_P:5 * TILE_P])
                nc.sync.dma_start(xr_sb[:, 3 * TILE_P:7 * TILE_P],
                                  x_d[:, 5 * TILE_P:9 * TILE_P])
            else:
                nc.sync.dma_start(xr_sb[:, 0:7 * TILE_P],
                                  x_d[:, 2 * TILE_P:9 * TILE_P])
            nc.sync.dma_start(xr_sb[:, 7 * TILE_P:15 * TILE_P],
                              x_d[:, 9 * TILE_P:17 * TILE_P])
            nc.scalar.dma_start(cc_sb[:, BB:BB + 2 * D],
                                cc_d[:, BB:BB + 2 * D])
            nc.scalar.dma_start(cc_sb[:, BB + 2 * D:CCW],
                                cc_d[:, BB + 2 * D:CCW])
            i_ap = cc_sb[:, D:D + TILE_P]

            def bias_pair(j):
                base = BB + ((2 * j) % N_BIAS) * D
                return cc_sb[:, base:base + PW]

            def w_ap(u, lo, hi):
                return cc_sb[64 * u:64 * u + F, lo:hi]

            # resident x~^T (bf16): xt_sb[64u+f, 128j+n] = x~[2j+u, n, f]
            xt_sb = cpool.tile([TILE_P, N_PAIR * TILE_P], mybir.dt.bfloat16)

            # prologue: the 16 transpose+mix matmuls use banks 0-1 of
            # pair 0's and pair 1's psum tiles as workspace (so each
            # pair's m3/m4 banks carry no WAR dependency on the late
            # transpose blocks), interleaved with the xt_sb copies.
            pair0 = psp.tile([TILE_P, PW], mybir.dt.float32,
                             name="pair", tag="pair")
            pair1 = psp.tile([TILE_P, PW], mybir.dt.float32,
                             name="pair", tag="pair")

            def tr_block(b4):
                # 4 transpose+mix matmuls into workspace banks, then one
                # bulk copy to xt_sb (ACT for even blocks, DVE for odd)
                wtile = pair0 if b4 < 2 else pair1
                wcol = 512 * (b4 % 2)
                for k in range(4):
                    pr = 4 * b4 + k
                    if wide0 and pr < 4:
                        src = at_x0[:, TILE_P * (pr + 1):TILE_P * (pr + 2)]
                    elif pr == 0:
                        src = at_x0[:, TILE_P:2 * TILE_P]
                    else:
                        src = xr_sb[:, 128 * (pr - 1):128 * pr]
                    nc.tensor.matmul(wtile[:, wcol + 128 * k:
                                           wcol + 128 * (k + 1)],
                                     src, at_ap, start=True, stop=True)
                if b4 % 2 == 0:
                    nc.scalar.copy(xt_sb[:, 512 * b4:512 * (b4 + 1)],
                                   wtile[:, wcol:wcol + 512])
                else:
                    nc.vector.tensor_copy(xt_sb[:, 512 * b4:512 * (b4 + 1)],
                                          wtile[:, wcol:wcol + 512])

            state = {"prev": None}

            def pair_body(j):
                bias = bias_pair(j)
                pair = (pair0 if j == 0 else pair1 if j == 1 else psp.tile(
                    [TILE_P, PW], mybir.dt.float32, name="pair", tag="pair"))
                lhs0 = xt_sb[0:64, 128 * j:128 * (j + 1)]
                lhs1 = xt_sb[64:128, 128 * j:128 * (j + 1)]
                nc.tensor.matmul(pair[:, 0:512], lhs0,
                                 w_ap(0, 0, 512), start=True, stop=True)
                nc.tensor.matmul(pair[:, 512:1024], lhs0,
                                 w_ap(0, 512, 1024), start=True, stop=True)
                nc.tensor.matmul(pair[:, 1024:1536], lhs1,
                                 w_ap(1, 0, 512), start=True, stop=True)
                nc.tensor.matmul(pair[:, 1536:2048], lhs1,
                                 w_ap(1, 512, 1024), start=True, stop=True)
                if inject:
                    nc.tensor.matmul(pair[:, 1536:2048], i_ap,
                                     bias[:, 1536:2048], start=False,
                                     stop=True, skip_group_check=True)
                o_t = opool.tile([TILE_P, PW], mybir.dt.bfloat16,
                                 name="o_t")
                # DVE fused drain+bias (PSUM 1x), ACT plain drain (1x)
                nc.vector.tensor_add(o_t[:, 0:xv], pair[:, 0:xv],
                                     bias[:, 0:xv])
                nc.scalar.copy(o_t[:, xv:PW], pair[:, xv:PW])
                if state["prev"] is not None:
                    _finish_pair(nc, out_d, *state["prev"], cfg)
                state["prev"] = (j, o_t, bias)

            # ramp: pair 0 runs between transpose blocks so its mains are
            # not queued behind the full transpose sweep on PE/ACT
            tr_block(0)
            tr_block(1)
            pair_body(0)
            if late23:
                with tc.tile_wait_until(0.004):
                    tr_block(2)
                    tr_block(3)
            else:
                tr_block(2)
                tr_block(3)
            for j in range(1, N_PAIR):
                pair_body(j)
            _finish_pair(nc, out_d, *state["prev"], cfg)
    nc.compile()
    return nc


def _finish_pair(nc, out_d, j, o_t, bias, cfg):
    # bias add for ACT's drained region: bf16 SBUF tensor_tensor (2x),
    # in place; then both stores.
    xv = cfg["XV"]
    hi = 1536 if cfg["INJECT"] else PW
    nc.vector.tensor_add(o_t[:, xv:hi], o_t[:, xv:hi], bias[:, xv:hi])
    if cfg["TAILSPLIT"] and j == N_PAIR - 1:
        for q in range(4):
            r0 = 256 * j + 128 * (q // 2)
            c0 = 512 * (q % 2)
            nc.sync.dma_start(out_d[r0:r0 + 128, c0:c0 + 512],
                              o_t[:, 512 * q:512 * (q + 1)])
        return
    nc.sync.dma_start(out_d[256 * j:256 * j + 128, :], o_t[:, 0:D])
    nc.sync.dma_start(out_d[256 * j + 128:256 * j + 256, :], o_t[:, D:PW])


def _host_constants(W_emb, b_emb, w_seg, b_seg):
    # sinusoidal positional encoding, float32, same formula as the reference
    pos = np.arange(S, dtype=np.float32)[:, None]
    div = np.exp(np.arange(0, D, 2, dtype=np.float32)
                 * (-np.log(10000.0) / D)).astype(np.float32)
    ang = pos * div
    pe = np.zeros((S, D), np.float32)
    pe[:, 0::2] = np.sin(ang)
    pe[:, 1::2] = np.cos(ang)

    bias = (pe + b_emb[None, :] * (np.float32(1.0) + w_seg.sum())
            + b_seg[0]).astype(np.float32)
    # rearrange to [128, 4*D]: column block j holds bias rows j*128..j*128+127
    bias_r = np.ascontiguousarray(
        bias.reshape(N_BIAS, TILE_P, D).transpose(1, 0, 2).reshape(
            TILE_P, N_BIAS * D)).astype(ml_dtypes.bfloat16)

    blk = np.eye(SEG, dtype=np.float32) + w_seg[:, None] * np.ones(
        (1, SEG), np.float32)
    at = np.kron(np.eye(TILE_P // SEG, dtype=np.float32), blk).astype(
        ml_dtypes.bfloat16)

    wb = np.vstack([W_emb, W_emb]).astype(ml_dtypes.bfloat16)
    ident = np.eye(TILE_P, dtype=np.float32).astype(ml_dtypes.bfloat16)
    # combined consts: [W2|I128|bias0..3] as [128, 5*D+128] bf16
    cc = np.ascontiguousarray(np.concatenate([wb, ident, bias_r], axis=1))
    return at, cc


def _prepare_in_maps(x, W_emb, b_emb, w_seg, b_seg):
    x = np.ascontiguousarray(np.asarray(x, dtype=np.float32))
    W_emb = np.asarray(W_emb, dtype=np.float32)
    b_emb = np.asarray(b_emb, dtype=np.float32)
    w_seg = np.asarray(w_seg, dtype=np.float32)
    b_seg = np.asarray(b_seg, dtype=np.float32)

    at, cc = _host_constants(W_emb, b_emb, w_seg, b_seg)

    in_maps = []
    for c in range(N_CORES):
        xs = x[c * B_LOC:(c + 1) * B_LOC].reshape(ROWS, F)
        # rearrange [32 tiles, 128 rows, F] -> [128, 32*F], bf16 staging
        xr = np.ascontiguousarray(
            xs.reshape(N_TILES, TILE_P, F).transpose(1, 0, 2).reshape(
                TILE_P, N_TILES * F)).astype(ml_dtypes.bfloat16)
        in_maps.append(
            {"x": np.ascontiguousarray(np.concatenate([at, xr], axis=1)),
             "cc": cc})
    return in_maps


def kernel(x, W_emb, b_emb, w_seg, b_seg):
    in_maps = _prepare_in_maps(x, W_emb, b_emb, w_seg, b_seg)

    global _NC_CACHE
    if _NC_CACHE is None:
        _NC_CACHE = _build_nc()

    res = run_bass_kernel_spmd(_NC_CACHE, in_maps,
                               core_ids=list(range(N_CORES)))
    out = np.concatenate(
        [np.asarray(res.results[c]["out"]).astype(np.float32).reshape(
            B_LOC, S, D) for c in range(N_CORES)], axis=0)
    return out


# revision 19
# speedup vs baseline: 1.0272x; 1.0205x over previous
"""BERT input representation kernel for 8 TRN2 NeuronCores.

Math (reference):
    x1  = x @ W_emb + b_emb                      # [B,S,D]
    seg = einsum('bnsd,s->bnd', x1.reshape(B,S/8,8,D), w_seg) + b_seg
    out = (x1.reshape(...) + seg[:,:,None,:]).reshape(B,S,D) + PE(S,D)

Folded form used here (exact algebra):
    out[b,s,:] = (A @ x[b])[s,:] @ W_emb + bias[s,:]
where A = I + blockdiag(ones(8,1) @ w_seg[None,:]) mixes rows within each
8-row segment, and bias[s,:] = PE[s,:] + b_emb*(1 + sum(w_seg)) + b_seg.

Sharding: pure data-parallel over batch; each of 8 cores handles 8
batches (4096 rows = 32 row-tiles of 128 rows = 16 tile-pair groups).

Schedule (v6):
  - output stored bf16 (host upcasts to f32): store traffic halves vs
    f32, 8.4 MiB/core written, ~2 MiB read
  - prologue: x loads in 3 sync-ring DMAs, W+identity then bias tiles
    on the scalar ring; the 16 transpose+segment-mix matmuls use banks
    0-1 of pair 0/1's PSUM tiles as workspace, with bulk PSUM->SBUF
    copies (alternating ACT/DVE) building the resident bf16 x~^T.
    Pair 0's body is emitted between transpose blocks 1 and 2, and
    blocks 2-3 carry a late scheduling hint (tile_wait_until), so the
    first store launches as early as possible.
  - steady loop per pair j: one [128,2048] f32 PSUM tile (4 banks,
    2 bufs = all of PSUM), 4 mains (start=True, FD=512).  Epilogue is
    split across both PSUM-capable engines: DVE does a fused
    drain+bias tensor_tensor on cols [0:XV) (PSUM 1x mode), ACT
    plain-drains [XV:2048) (1x), and DVE adds bias there as a bf16
    SBUF tensor_tensor (2x packed mode).  The DVE add for pair j is
    emitted after pair j+1's fused op (software pipelining) so DVE
    never idles waiting for ACT.  Steady cadence ~1.7 us/pair,
    ACT/DVE-balanced (the f32 PSUM drain at 1 elem/cycle/lane is the
    hard floor on TRN2; matmul cannot write bf16 PSUM here).
  - two 256 KiB bf16 stores per pair on the sync HWDGE ring
"""

import sys

if "/opt/trn_rl_repo" not in sys.path:
    sys.path.insert(0, "/opt/trn_rl_repo")

import ml_dtypes
import numpy as np

import concourse.bacc as bacc
import concourse.mybir as mybir
import concourse.tile as tile
from concourse.bass_utils import run_bass_kernel_spmd

B, S, F, D, SEG = 64, 512, 64, 1024, 8
N_CORES = 8
B_LOC = B // N_CORES          # batches per core
ROWS = B_LOC * S              # 4096 rows per core
TILE_P = 128                  # rows per tile
N_TILES = ROWS // TILE_P      # 32
N_PAIR = N_TILES // 2         # 16 tile-pairs
N_BIAS = S // TILE_P          # 4 distinct bias row-tiles
PW = 2 * D                    # 2048 cols per pair psum tile
XV = 576                      # DVE fused drain+bias covers cols [0:XV)

_NC_CACHE = None
DEFAULT_CFG = {"XV": XV, "INJECT": False, "LATE23": True, "XSPLIT": False,
               "WIDE0": False, "TAILSPLIT": False, "OBUFS": 4, "GPS": 0,
               "MM1024": False}


def _build_nc(cfg=None):
    cfg = dict(DEFAULT_CFG, **(cfg or {}))
    xv, inject = cfg["XV"], cfg["INJECT"]
    late23, xsplit = cfg["LATE23"], cfg["XSPLIT"]
    wide0, tailsplit = cfg["WIDE0"], cfg["TAILSPLIT"]
    obufs = cfg["OBUFS"]
    mm1024 = cfg["MM1024"]
    nc = bacc.Bacc("TRN2", target_bir_lowering=False, debug=False,
                   num_devices=N_CORES)
    # x pre-rearranged on host (layout + cast to bf16):
    # xr[p, i*F:(i+1)*F] = x[i*128+p]; cols [0:128] = A^T
    x_d = nc.declare_dram_parameter("x", [TILE_P, TILE_P + N_TILES * F],
                                    mybir.dt.bfloat16, isOutput=False)
    # combined constants [128, 5120]: cols [0:1024]=W stacked twice
    # (partitions 0-63 and 64-127 both hold W) | [1024:5120]=bias0..3
    cc_d = nc.declare_dram_parameter("cc", [TILE_P, 5 * D + TILE_P],
                                     mybir.dt.bfloat16, isOutput=False)
    out_d = nc.declare_dram_parameter("out", [ROWS, D], mybir.dt.bfloat16,
                                      isOutput=True)

    with tile.TileContext(nc) as tc:
        with (
            tc.tile_pool(name="const", bufs=1) as cpool,
            tc.tile_pool(name="outp", bufs=obufs) as opool,
            tc.tile_pool(name="ps", bufs=2, space="PSUM") as psp,
        ):
            # loads: sync ring carries A^T+x then the stores; scalar ring
            # carries W and the bias tiles in need-order.
            n0 = 5 if wide0 else 2     # cols of x_d in the first DMA
            at_x0 = cpool.tile([TILE_P, n0 * TILE_P], mybir.dt.bfloat16)
            nc.sync.dma_start(at_x0[:], x_d[:, 0:n0 * TILE_P])
            at_ap = at_x0[:, 0:TILE_P]
            CCW = 5 * D + TILE_P
            BB = D + TILE_P            # bias block base in cc
            cc_sb = cpool.tile([TILE_P, CCW], mybir.dt.bfloat16)
            nc.scalar.dma_start(cc_sb[:, 0:BB], cc_d[:, 0:BB])
            xr_sb = cpool.tile([TILE_P, (N_PAIR - 1) * TILE_P],
                               mybir.dt.bfloat16)
            if wide0:
                nc.sync.dma_start(xr_sb[:, 3 * TILE_P:7 * TILE_P],
                                  x_d[:, 5 * TILE_P:9 * TILE_P])
            elif xsplit:
                nc.sync.dma_start(xr_sb[:, 0:3 * TILE_P],
                                  x_d[:, 2 * TILE_P:5 * TILE_P])
                nc.sync.dma_start(xr_sb[:, 3 * TILE_P:7 * TILE_P],
                                  x_d[:, 5 * TILE_P:9 * TILE_P])
            else:
                nc.sync.dma_start(xr_sb[:, 0:7 * TILE_P],
                                  x_d[:, 2 * TILE_P:9 * TILE_P])
            nc.sync.dma_start(xr_sb[:, 7 * TILE_P:15 * TILE_P],
                              x_d[:, 9 * TILE_P:17 * TILE_P])
            nc.scalar.dma_start(cc_sb[:, BB:BB + 2 * D],
                                cc_d[:, BB:BB + 2 * D])
            nc.scalar.dma_start(cc_sb[:, BB + 2 * D:CCW],
                                cc_d[:, BB + 2 * D:CCW])
            i_ap = cc_sb[:, D:D + TILE_P]

            def bias_pair(j):
                base = BB + ((2 * j) % N_BIAS) * D
                return cc_sb[:, base:base + PW]

            def w_ap(u, lo, hi):
                return cc_sb[64 * u:64 * u + F, lo:hi]

            # resident x~^T (bf16): xt_sb[64u+f, 128j+n] = x~[2j+u, n, f]
            xt_sb = cpool.tile([TILE_P, N_PAIR * TILE_P], mybir.dt.bfloat16)

            # prologue: the 16 transpose+mix matmuls use banks 0-1 of
            # pair 0's and pair 1's psum tiles as workspace (so each
            # pair's m3/m4 banks carry no WAR dependency on the late
            # transpose blocks), interleaved with the xt_sb copies.
            pair0 = psp.tile([TILE_P, PW], mybir.dt.float32,
                             name="pair", tag="pair")
            pair1 = psp.tile([TILE_P, PW], mybir.dt.float32,
                             name="pair", tag="pair")

            def tr_block(b4):
                # 4 transpose+mix matmuls into workspace banks, then one
                # bulk copy to xt_sb (ACT for even blocks, DVE for odd)
                wtile = pair0 if b4 < 2 else pair1
                wcol = 512 * (b4 % 2)
                for k in range(4):
                    pr = 4 * b4 + k
                    if wide0 and pr < 4:
                        src = at_x0[:, TILE_P * (pr + 1):TILE_P * (pr + 2)]
                    elif pr == 0:
                        src = at_x0[:, TILE_P:2 * TILE_P]
                    else:
                        src = xr_sb[:, 128 * (pr - 1):128 * pr]
                    nc.tensor.matmul(wtile[:, wcol + 128 * k:
                                           wcol + 128 * (k + 1)],
                                     src, at_ap, start=True, stop=True)
                if b4 % 2 == 0:
                    nc.scalar.copy(xt_sb[:, 512 * b4:512 * (b4 + 1)],
                                   wtile[:, wcol:wcol + 512])
                else:
                    nc.vector.tensor_copy(xt_sb[:, 512 * b4:512 * (b4 + 1)],
                                          wtile[:, wcol:wcol + 512])

            state = {"prev": None}

            def pair_body(j):
                bias = bias_pair(j)
                pair = (pair0 if j == 0 else pair1 if j == 1 else psp.tile(
                    [TILE_P, PW], mybir.dt.float32, name="pair", tag="pair"))
                lhs0 = xt_sb[0:64, 128 * j:128 * (j + 1)]
                lhs1 = xt_sb[64:128, 128 * j:128 * (j + 1)]
                if mm1024:
                    nc.tensor.matmul(pair[:, 0:1024], lhs0,
                                     w_ap(0, 0, 1024), start=True, stop=True)
                    nc.tensor.matmul(pair[:, 1024:2048], lhs1,
                                     w_ap(1, 0, 1024), start=True, stop=True)
                else:
                    nc.tensor.matmul(pair[:, 0:512], lhs0,
                                     w_ap(0, 0, 512), start=True, stop=True)
                    nc.tensor.matmul(pair[:, 512:1024], lhs0,
                                     w_ap(0, 512, 1024),
                                     start=True, stop=True)
                    nc.tensor.matmul(pair[:, 1024:1536], lhs1,
                                     w_ap(1, 0, 512), start=True, stop=True)
                    nc.tensor.matmul(pair[:, 1536:2048], lhs1,
                                     w_ap(1, 512, 1024),
                                     start=True, stop=True)
                if inject:
                    nc.tensor.matmul(pair[:, 1536:2048], i_ap,
                                     bias[:, 1536:2048], start=False,
                                     stop=True, skip_group_check=True)
                o_t = opool.tile([TILE_P, PW], mybir.dt.bfloat16,
                                 name="o_t")
                # DVE fused drain+bias (PSUM 1x), ACT plain drain (1x)
                nc.vector.tensor_add(o_t[:, 0:xv], pair[:, 0:xv],
                                     bias[:, 0:xv])
                nc.scalar.copy(o_t[:, xv:PW], pair[:, xv:PW])
                if state["prev"] is not None:
                    _finish_pair(nc, out_d, *state["prev"], cfg)
                state["prev"] = (j, o_t, bias)

            # ramp: pair 0 runs between transpose blocks so its mains are
            # not queued behind the full transpose sweep on PE/ACT
            tr_block(0)
            tr_block(1)
            pair_body(0)
            if late23:
                with tc.tile_wait_until(0.004):
                    tr_block(2)
                    tr_block(3)
            else:
                tr_block(2)
                tr_block(3)
            for j in range(1, N_PAIR):
                pair_body(j)
            _finish_pair(nc, out_d, *state["prev"], cfg)
    nc.compile()
    return nc


def _finish_pair(nc, out_d, j, o_t, bias, cfg):
    # bias add for ACT's drained region: bf16 SBUF tensor_tensor (2x),
    # in place; then both stores.
    xv = cfg["XV"]
    hi = 1536 if cfg["INJECT"] else PW
    gps = cfg["GPS"]
    if gps:
        nc.vector.tensor_add(o_t[:, xv:hi - gps], o_t[:, xv:hi - gps],
                             bias[:, xv:hi - gps])
        nc.gpsimd.tensor_add(o_t[:, hi - gps:hi], o_t[:, hi - gps:hi],
                             bias[:, hi - gps:hi])
    else:
        nc.vector.tensor_add(o_t[:, xv:hi], o_t[:, xv:hi], bias[:, xv:hi])
    if cfg["TAILSPLIT"] and j == N_PAIR - 1:
        for q in range(4):
            r0 = 256 * j + 128 * (q // 2)
            c0 = 512 * (q % 2)
            nc.sync.dma_start(out_d[r0:r0 + 128, c0:c0 + 512],
                              o_t[:, 512 * q:512 * (q + 1)])
        return
    nc.sync.dma_start(out_d[256 * j:256 * j + 128, :], o_t[:, 0:D])
    nc.sync.dma_start(out_d[256 * j + 128:256 * j + 256, :], o_t[:, D:PW])


def _host_constants(W_emb, b_emb, w_seg, b_seg):
    # sinusoidal positional encoding, float32, same formula as the reference
    pos = np.arange(S, dtype=np.float32)[:, None]
    div = np.exp(np.arange(0, D, 2, dtype=np.float32)
                 * (-np.log(10000.0) / D)).astype(np.float32)
    ang = pos * div
    pe = np.zeros((S, D), np.float32)
    pe[:, 0::2] = np.sin(ang)
    pe[:, 1::2] = np.cos(ang)

    bias = (pe + b_emb[None, :] * (np.float32(1.0) + w_seg.sum())
            + b_seg[0]).astype(np.float32)
    # rearrange to [128, 4*D]: column block j holds bias rows j*128..j*128+127
    bias_r = np.ascontiguousarray(
        bias.reshape(N_BIAS, TILE_P, D).transpose(1, 0, 2).reshape(
            TILE_P, N_BIAS * D)).astype(ml_dtypes.bfloat16)

    blk = np.eye(SEG, dtype=np.float32) + w_seg[:, None] * np.ones(
        (1, SEG), np.float32)
    at = np.kron(np.eye(TILE_P // SEG, dtype=np.float32), blk).astype(
        ml_dtypes.bfloat16)

    wb = np.vstack([W_emb, W_emb]).astype(ml_dtypes.bfloat16)
    ident = np.eye(TILE_P, dtype=np.float32).astype(ml_dtypes.bfloat16)
    # combined consts: [W2|I128|bias0..3] as [128, 5*D+128] bf16
    cc = np.ascontiguousarray(np.concatenate([wb, ident, bias_r], axis=1))
    return at, cc


def _prepare_in_maps(x, W_emb, b_emb, w_seg, b_seg):
    x = np.ascontiguousarray(np.asarray(x, dtype=np.float32))
    W_emb = np.asarray(W_emb, dtype=np.float32)
    b_emb = np.asarray(b_emb, dtype=np.float32)
    w_seg = np.asarray(w_seg, dtype=np.float32)
    b_seg = np.asarray(b_seg, dtype=np.float32)

    at, cc = _host_constants(W_emb, b_emb, w_seg, b_seg)

    in_maps = []
    for c in range(N_CORES):
        xs = x[c * B_LOC:(c + 1) * B_LOC].reshape(ROWS, F)
        # rearrange [32 tiles, 128 rows, F] -> [128, 32*F], bf16 staging
        xr = np.ascontiguousarray(
            xs.reshape(N_TILES, TILE_P, F).transpose(1, 0, 2).reshape(
                TILE_P, N_TILES * F)).astype(ml_dtypes.bfloat16)
        in_maps.append(
            {"x": np.ascontiguousarray(np.concatenate([at, xr], axis=1)),
             "cc": cc})
    return in_maps


def kernel(x, W_emb, b_emb, w_seg, b_seg):
    in_maps = _prepare_in_maps(x, W_emb, b_emb, w_seg, b_seg)

    global _NC_CACHE
    if _NC_CACHE is None:
        _NC_CACHE = _build_nc()

    res = run_bass_kernel_spmd(_NC_CACHE, in_maps,
                               core_ids=list(range(N_CORES)))
    out = np.concatenate(
        [np.asarray(res.results[c]["out"]).astype(np.float32).reshape(
            B_LOC, S, D) for c in range(N_CORES)], axis=0)
    return out


# revision 21
# speedup vs baseline: 1.0333x; 1.0059x over previous
"""BERT input representation kernel for 8 TRN2 NeuronCores.

Math (reference):
    x1  = x @ W_emb + b_emb                      # [B,S,D]
    seg = einsum('bnsd,s->bnd', x1.reshape(B,S/8,8,D), w_seg) + b_seg
    out = (x1.reshape(...) + seg[:,:,None,:]).reshape(B,S,D) + PE(S,D)

Folded form used here (exact algebra):
    out[b,s,:] = (A @ x[b])[s,:] @ W_emb + bias[s,:]
where A = I + blockdiag(ones(8,1) @ w_seg[None,:]) mixes rows within each
8-row segment, and bias[s,:] = PE[s,:] + b_emb*(1 + sum(w_seg)) + b_seg.

Sharding: pure data-parallel over batch; each of 8 cores handles 8
batches (4096 rows = 32 row-tiles of 128 rows = 16 tile-pair groups).

Schedule (v6):
  - output stored bf16 (host upcasts to f32): store traffic halves vs
    f32, 8.4 MiB/core written, ~2 MiB read
  - prologue: x loads in 3 sync-ring DMAs, W+identity then bias tiles
    on the scalar ring; the 16 transpose+segment-mix matmuls use banks
    0-1 of pair 0/1's PSUM tiles as workspace, with bulk PSUM->SBUF
    copies (alternating ACT/DVE) building the resident bf16 x~^T.
    Pair 0's body is emitted between transpose blocks 1 and 2, and
    blocks 2-3 carry a late scheduling hint (tile_wait_until), so the
    first store launches as early as possible.
  - steady loop per pair j: one [128,2048] f32 PSUM tile (4 banks,
    2 bufs = all of PSUM), 4 mains (start=True, FD=512).  Epilogue is
    split across both PSUM-capable engines: DVE does a fused
    drain+bias tensor_tensor on cols [0:XV) (PSUM 1x mode), ACT
    plain-drains [XV:2048) (1x), and DVE adds bias there as a bf16
    SBUF tensor_tensor (2x packed mode).  The DVE add for pair j is
    emitted after pair j+1's fused op (software pipelining) so DVE
    never idles waiting for ACT.  Steady cadence ~1.7 us/pair,
    ACT/DVE-balanced (the f32 PSUM drain at 1 elem/cycle/lane is the
    hard floor on TRN2; matmul cannot write bf16 PSUM here).
  - two 256 KiB bf16 stores per pair on the sync HWDGE ring; the
    final pair's epilogue runs as one fused DVE op with its stores on
    the otherwise-idle scalar ring (shortest tail chain before the
    last store's HBM write receipt, which gates the NEFF end barrier)
"""

import sys

if "/opt/trn_rl_repo" not in sys.path:
    sys.path.insert(0, "/opt/trn_rl_repo")

import ml_dtypes
import numpy as np

import concourse.bacc as bacc
import concourse.mybir as mybir
import concourse.tile as tile
from concourse.bass_utils import run_bass_kernel_spmd

B, S, F, D, SEG = 64, 512, 64, 1024, 8
N_CORES = 8
B_LOC = B // N_CORES          # batches per core
ROWS = B_LOC * S              # 4096 rows per core
TILE_P = 128                  # rows per tile
N_TILES = ROWS // TILE_P      # 32
N_PAIR = N_TILES // 2         # 16 tile-pairs
N_BIAS = S // TILE_P          # 4 distinct bias row-tiles
PW = 2 * D                    # 2048 cols per pair psum tile
XV = 576                      # DVE fused drain+bias covers cols [0:XV)

_NC_CACHE = None
DEFAULT_CFG = {"XV": XV, "INJECT": False, "LATE23": True, "XSPLIT": False,
               "WIDE0": False, "TAILSPLIT": False, "OBUFS": 4, "GPS": 0,
               "MM1024": False, "TAILFUSE": True, "STEADYHINT": False}


def _build_nc(cfg=None):
    cfg = dict(DEFAULT_CFG, **(cfg or {}))
    xv, inject = cfg["XV"], cfg["INJECT"]
    late23, xsplit = cfg["LATE23"], cfg["XSPLIT"]
    wide0, tailsplit = cfg["WIDE0"], cfg["TAILSPLIT"]
    obufs = cfg["OBUFS"]
    mm1024 = cfg["MM1024"]
    tailfuse = cfg["TAILFUSE"]
    steadyhint = cfg["STEADYHINT"]
    nc = bacc.Bacc("TRN2", target_bir_lowering=False, debug=False,
                   num_devices=N_CORES)
    # x pre-rearranged on host (layout + cast to bf16):
    # xr[p, i*F:(i+1)*F] = x[i*128+p]; cols [0:128] = A^T
    x_d = nc.declare_dram_parameter("x", [TILE_P, TILE_P + N_TILES * F],
                                    mybir.dt.bfloat16, isOutput=False)
    # combined constants [128, 5120]: cols [0:1024]=W stacked twice
    # (partitions 0-63 and 64-127 both hold W) | [1024:5120]=bias0..3
    cc_d = nc.declare_dram_parameter("cc", [TILE_P, 5 * D + TILE_P],
                                     mybir.dt.bfloat16, isOutput=False)
    out_d = nc.declare_dram_parameter("out", [ROWS, D], mybir.dt.bfloat16,
                                      isOutput=True)

    with tile.TileContext(nc) as tc:
        with (
            tc.tile_pool(name="const", bufs=1) as cpool,
            tc.tile_pool(name="outp", bufs=obufs) as opool,
            tc.tile_pool(name="ps", bufs=2, space="PSUM") as psp,
        ):
            # loads: sync ring carries A^T+x then the stores; scalar ring
            # carries W and the bias tiles in need-order.
            n0 = 5 if wide0 else 2     # cols of x_d in the first DMA
            at_x0 = cpool.tile([TILE_P, n0 * TILE_P], mybir.dt.bfloat16)
            nc.sync.dma_start(at_x0[:], x_d[:, 0:n0 * TILE_P])
            at_ap = at_x0[:, 0:TILE_P]
            CCW = 5 * D + TILE_P
            BB = D + TILE_P            # bias block base in cc
            cc_sb = cpool.tile([TILE_P, CCW], mybir.dt.bfloat16)
            nc.scalar.dma_start(cc_sb[:, 0:BB], cc_d[:, 0:BB])
            xr_sb = cpool.tile([TILE_P, (N_PAIR - 1) * TILE_P],
                               mybir.dt.bfloat16)
            if wide0:
                nc.sync.dma_start(xr_sb[:, 3 * TILE_P:7 * TILE_P],
                                  x_d[:, 5 * TILE_P:9 * TILE_P])
            elif xsplit:
                nc.sync.dma_start(xr_sb[:, 0:3 * TILE_P],
                                  x_d[:, 2 * TILE_P:5 * TILE_P])
                nc.sync.dma_start(xr_sb[:, 3 * TILE_P:7 * TILE_P],
                                  x_d[:, 5 * TILE_P:9 * TILE_P])
            else:
                nc.sync.dma_start(xr_sb[:, 0:7 * TILE_P],
                                  x_d[:, 2 * TILE_P:9 * TILE_P])
            nc.sync.dma_start(xr_sb[:, 7 * TILE_P:15 * TILE_P],
                              x_d[:, 9 * TILE_P:17 * TILE_P])
            nc.scalar.dma_start(cc_sb[:, BB:BB + 2 * D],
                                cc_d[:, BB:BB + 2 * D])
            nc.scalar.dma_start(cc_sb[:, BB + 2 * D:CCW],
                                cc_d[:, BB + 2 * D:CCW])
            i_ap = cc_sb[:, D:D + TILE_P]

            def bias_pair(j):
                base = BB + ((2 * j) % N_BIAS) * D
                return cc_sb[:, base:base + PW]

            def w_ap(u, lo, hi):
                return cc_sb[64 * u:64 * u + F, lo:hi]

            # resident x~^T (bf16): xt_sb[64u+f, 128j+n] = x~[2j+u, n, f]
            xt_sb = cpool.tile([TILE_P, N_PAIR * TILE_P], mybir.dt.bfloat16)

            # prologue: the 16 transpose+mix matmuls use banks 0-1 of
            # pair 0's and pair 1's psum tiles as workspace (so each
            # pair's m3/m4 banks carry no WAR dependency on the late
            # transpose blocks), interleaved with the xt_sb copies.
            pair0 = psp.tile([TILE_P, PW], mybir.dt.float32,
                             name="pair", tag="pair")
            pair1 = psp.tile([TILE_P, PW], mybir.dt.float32,
                             name="pair", tag="pair")

            def tr_block(b4):
                # 4 transpose+mix matmuls into workspace banks, then one
                # bulk copy to xt_sb (ACT for even blocks, DVE for odd)
                wtile = pair0 if b4 < 2 else pair1
                wcol = 512 * (b4 % 2)
                for k in range(4):
                    pr = 4 * b4 + k
                    if wide0 and pr < 4:
                        src = at_x0[:, TILE_P * (pr + 1):TILE_P * (pr + 2)]
                    elif pr == 0:
                        src = at_x0[:, TILE_P:2 * TILE_P]
                    else:
                        src = xr_sb[:, 128 * (pr - 1):128 * pr]
                    nc.tensor.matmul(wtile[:, wcol + 128 * k:
                                           wcol + 128 * (k + 1)],
                                     src, at_ap, start=True, stop=True)
                if b4 % 2 == 0:
                    nc.scalar.copy(xt_sb[:, 512 * b4:512 * (b4 + 1)],
                                   wtile[:, wcol:wcol + 512])
                else:
                    nc.vector.tensor_copy(xt_sb[:, 512 * b4:512 * (b4 + 1)],
                                          wtile[:, wcol:wcol + 512])

            state = {"prev": None}

            def pair_body(j):
                bias = bias_pair(j)
                pair = (pair0 if j == 0 else pair1 if j == 1 else psp.tile(
                    [TILE_P, PW], mybir.dt.float32, name="pair", tag="pair"))
                lhs0 = xt_sb[0:64, 128 * j:128 * (j + 1)]
                lhs1 = xt_sb[64:128, 128 * j:128 * (j + 1)]
                if mm1024:
                    nc.tensor.matmul(pair[:, 0:1024], lhs0,
                                     w_ap(0, 0, 1024), start=True, stop=True)
                    nc.tensor.matmul(pair[:, 1024:2048], lhs1,
                                     w_ap(1, 0, 1024), start=True, stop=True)
                else:
                    nc.tensor.matmul(pair[:, 0:512], lhs0,
                                     w_ap(0, 0, 512), start=True, stop=True)
                    nc.tensor.matmul(pair[:, 512:1024], lhs0,
                                     w_ap(0, 512, 1024),
                                     start=True, stop=True)
                    nc.tensor.matmul(pair[:, 1024:1536], lhs1,
                                     w_ap(1, 0, 512), start=True, stop=True)
                    nc.tensor.matmul(pair[:, 1536:2048], lhs1,
                                     w_ap(1, 512, 1024),
                                     start=True, stop=True)
                if inject:
                    nc.tensor.matmul(pair[:, 1536:2048], i_ap,
                                     bias[:, 1536:2048], start=False,
                                     stop=True, skip_group_check=True)
                o_t = opool.tile([TILE_P, PW], mybir.dt.bfloat16,
                                 name="o_t")
                if tailfuse and j == N_PAIR - 1:
                    # final pair: single fused DVE op, stores on the idle
                    # scalar ring -> shortest possible tail chain
                    nc.vector.tensor_add(o_t[:], pair[:], bias[:])
                    nc.scalar.dma_start(out_d[256 * j:256 * j + 128, :],
                                        o_t[:, 0:D])
                    nc.scalar.dma_start(
                        out_d[256 * j + 128:256 * j + 256, :],
                        o_t[:, D:PW])
                    if state["prev"] is not None:
                        _finish_pair(nc, out_d, *state["prev"], cfg)
                    state["prev"] = None
                    return
                # DVE fused drain+bias (PSUM 1x), ACT plain drain (1x)
                nc.vector.tensor_add(o_t[:, 0:xv], pair[:, 0:xv],
                                     bias[:, 0:xv])
                nc.scalar.copy(o_t[:, xv:PW], pair[:, xv:PW])
                if state["prev"] is not None:
                    _finish_pair(nc, out_d, *state["prev"], cfg)
                state["prev"] = (j, o_t, bias)

            # ramp: pair 0 runs between transpose blocks so its mains are
            # not queued behind the full transpose sweep on PE/ACT
            tr_block(0)
            tr_block(1)
            pair_body(0)
            if late23:
                with tc.tile_wait_until(0.004):
                    tr_block(2)
                    tr_block(3)
            else:
                tr_block(2)
                tr_block(3)
            for j in range(1, N_PAIR):
                if steadyhint:
                    with tc.tile_wait_until(0.004 + 0.0017 * j):
                        pair_body(j)
                else:
                    pair_body(j)
            if state["prev"] is not None:
                _finish_pair(nc, out_d, *state["prev"], cfg)
    nc.compile()
    return nc


def _finish_pair(nc, out_d, j, o_t, bias, cfg):
    # bias add for ACT's drained region: bf16 SBUF tensor_tensor (2x),
    # in place; then both stores.
    xv = cfg["XV"]
    hi = 1536 if cfg["INJECT"] else PW
    gps = cfg["GPS"]
    if gps:
        nc.vector.tensor_add(o_t[:, xv:hi - gps], o_t[:, xv:hi - gps],
                             bias[:, xv:hi - gps])
        nc.gpsimd.tensor_add(o_t[:, hi - gps:hi], o_t[:, hi - gps:hi],
                             bias[:, hi - gps:hi])
    else:
        nc.vector.tensor_add(o_t[:, xv:hi], o_t[:, xv:hi], bias[:, xv:hi])
    if cfg["TAILSPLIT"] and j == N_PAIR - 1:
        for q in range(4):
            r0 = 256 * j + 128 * (q // 2)
            c0 = 512 * (q % 2)
            nc.sync.dma_start(out_d[r0:r0 + 128, c0:c0 + 512],
                              o_t[:, 512 * q:512 * (q + 1)])
        return
    nc.sync.dma_start(out_d[256 * j:256 * j + 128, :], o_t[:, 0:D])
    nc.sync.dma_start(out_d[256 * j + 128:256 * j + 256, :], o_t[:, D:PW])


def _host_constants(W_emb, b_emb, w_seg, b_seg):
    # sinusoidal positional encoding, float32, same formula as the reference
    pos = np.arange(S, dtype=np.float32)[:, None]
    div = np.exp(np.arange(0, D, 2, dtype=np.float32)
                 * (-np.log(10000.0) / D)).astype(np.float32)
    ang = pos * div
    pe = np.zeros((S, D), np.float32)
    pe[:, 0::2] = np.sin(ang)
    pe[:, 1::2] = np.cos(ang)

    bias = (pe + b_emb[None, :] * (np.float32(1.0) + w_seg.sum())
            + b_seg[0]).astype(np.float32)
    # rearrange to [128, 4*D]: column block j holds bias rows j*128..j*128+127
    bias_r = np.ascontiguousarray(
        bias.reshape(N_BIAS, TILE_P, D).transpose(1, 0, 2).reshape(
            TILE_P, N_BIAS * D)).astype(ml_dtypes.bfloat16)

    blk = np.eye(SEG, dtype=np.float32) + w_seg[:, None] * np.ones(
        (1, SEG), np.float32)
    at = np.kron(np.eye(TILE_P // SEG, dtype=np.float32), blk).astype(
        ml_dtypes.bfloat16)

    wb = np.vstack([W_emb, W_emb]).astype(ml_dtypes.bfloat16)
    ident = np.eye(TILE_P, dtype=np.float32).astype(ml_dtypes.bfloat16)
    # combined consts: [W2|I128|bias0..3] as [128, 5*D+128] bf16
    cc = np.ascontiguousarray(np.concatenate([wb, ident, bias_r], axis=1))
    return at, cc


def _prepare_in_maps(x, W_emb, b_emb, w_seg, b_seg):
    x = np.ascontiguousarray(np.asarray(x, dtype=np.float32))
    W_emb = np.asarray(W_emb, dtype=np.float32)
    b_emb = np.asarray(b_emb, dtype=np.float32)
    w_seg = np.asarray(w_seg, dtype=np.float32)
    b_seg = np.asarray(b_seg, dtype=np.float32)

    at, cc = _host_constants(W_emb, b_emb, w_seg, b_seg)

    in_maps = []
    for c in range(N_CORES):
        xs = x[c * B_LOC:(c + 1) * B_LOC].reshape(ROWS, F)
        # rearrange [32 tiles, 128 rows, F] -> [128, 32*F], bf16 staging
        xr = np.ascontiguousarray(
            xs.reshape(N_TILES, TILE_P, F).transpose(1, 0, 2).reshape(
                TILE_P, N_TILES * F)).astype(ml_dtypes.bfloat16)
        in_maps.append(
            {"x": np.ascontiguousarray(np.concatenate([at, xr], axis=1)),
             "cc": cc})
    return in_maps


def kernel(x, W_emb, b_emb, w_seg, b_seg):
    in_maps = _prepare_in_maps(x, W_emb, b_emb, w_seg, b_seg)

    global _NC_CACHE
    if _NC_CACHE is None:
        _NC_CACHE = _build_nc()

    res = run_bass_kernel_spmd(_NC_CACHE, in_maps,
                               core_ids=list(range(N_CORES)))
    out = np.concatenate(
        [np.asarray(res.results[c]["out"]).astype(np.float32).reshape(
            B_LOC, S, D) for c in range(N_CORES)], axis=0)
    return out


# revision 23
# speedup vs baseline: 1.0565x; 1.0225x over previous
"""BERT input representation kernel for 8 TRN2 NeuronCores.

Math (reference):
    x1  = x @ W_emb + b_emb                      # [B,S,D]
    seg = einsum('bnsd,s->bnd', x1.reshape(B,S/8,8,D), w_seg) + b_seg
    out = (x1.reshape(...) + seg[:,:,None,:]).reshape(B,S,D) + PE(S,D)

Folded form used here (exact algebra):
    out[b,s,:] = (A @ x[b])[s,:] @ W_emb + bias[s,:]
where A = I + blockdiag(ones(8,1) @ w_seg[None,:]) mixes rows within each
8-row segment, and bias[s,:] = PE[s,:] + b_emb*(1 + sum(w_seg)) + b_seg.

Sharding: pure data-parallel over batch; each of 8 cores handles 8
batches (4096 rows = 32 row-tiles of 128 rows = 16 tile-pair groups).

Schedule (v6):
  - output stored bf16 (host upcasts to f32): store traffic halves vs
    f32, 8.4 MiB/core written, ~2 MiB read
  - prologue: the first sync-ring DMA carries A^T + x for pairs 0-7
    (288 KB; its completion receipt gates the whole ramp, and is only
    ~0.4 us later than a minimal load), a second carries pairs 8-15;
    W+identity then bias tiles go on the scalar ring; the 16 transpose+segment-mix matmuls use banks
    0-1 of pair 0/1's PSUM tiles as workspace, with bulk PSUM->SBUF
    copies (alternating ACT/DVE) building the resident bf16 x~^T.
    Pair 0's body is emitted between transpose blocks 1 and 2, and
    blocks 2-3 carry a late scheduling hint (tile_wait_until), so the
    first store launches as early as possible.
  - steady loop per pair j: one [128,2048] f32 PSUM tile (4 banks,
    2 bufs = all of PSUM), 4 mains (start=True, FD=512).  Epilogue is
    split across both PSUM-capable engines: DVE does a fused
    drain+bias tensor_tensor on cols [0:XV) (PSUM 1x mode), ACT
    plain-drains [XV:2048) (1x), and DVE adds bias there as a bf16
    SBUF tensor_tensor (2x packed mode).  The DVE add for pair j is
    emitted after pair j+1's fused op (software pipelining) so DVE
    never idles waiting for ACT.  Steady cadence ~1.7 us/pair,
    ACT/DVE-balanced (the f32 PSUM drain at 1 elem/cycle/lane is the
    hard floor on TRN2; matmul cannot write bf16 PSUM here).
  - two 256 KiB bf16 stores per pair on the sync HWDGE ring; the
    final pair's epilogue runs as one fused DVE op with its stores on
    the otherwise-idle scalar ring (shortest tail chain before the
    last store's HBM write receipt, which gates the NEFF end barrier)
"""

import sys

if "/opt/trn_rl_repo" not in sys.path:
    sys.path.insert(0, "/opt/trn_rl_repo")

import ml_dtypes
import numpy as np

import concourse.bacc as bacc
import concourse.mybir as mybir
import concourse.tile as tile
from concourse.bass_utils import run_bass_kernel_spmd

B, S, F, D, SEG = 64, 512, 64, 1024, 8
N_CORES = 8
B_LOC = B // N_CORES          # batches per core
ROWS = B_LOC * S              # 4096 rows per core
TILE_P = 128                  # rows per tile
N_TILES = ROWS // TILE_P      # 32
N_PAIR = N_TILES // 2         # 16 tile-pairs
N_BIAS = S // TILE_P          # 4 distinct bias row-tiles
PW = 2 * D                    # 2048 cols per pair psum tile
XV = 576                      # DVE fused drain+bias covers cols [0:XV)

_NC_CACHE = None
DEFAULT_CFG = {"XV": XV, "INJECT": False, "LATE23": True, "XSPLIT": False,
               "WIDE0": False, "TAILSPLIT": False, "OBUFS": 4, "GPS": 0,
               "MM1024": False, "TAILFUSE": True, "STEADYHINT": False,
               "WIDE8": True}


def _build_nc(cfg=None):
    cfg = dict(DEFAULT_CFG, **(cfg or {}))
    xv, inject = cfg["XV"], cfg["INJECT"]
    late23, xsplit = cfg["LATE23"], cfg["XSPLIT"]
    wide0, tailsplit = cfg["WIDE0"], cfg["TAILSPLIT"]
    obufs = cfg["OBUFS"]
    mm1024 = cfg["MM1024"]
    tailfuse = cfg["TAILFUSE"]
    steadyhint = cfg["STEADYHINT"]
    wide8 = cfg["WIDE8"]
    nc = bacc.Bacc("TRN2", target_bir_lowering=False, debug=False,
                   num_devices=N_CORES)
    # x pre-rearranged on host (layout + cast to bf16):
    # xr[p, i*F:(i+1)*F] = x[i*128+p]; cols [0:128] = A^T
    x_d = nc.declare_dram_parameter("x", [TILE_P, TILE_P + N_TILES * F],
                                    mybir.dt.bfloat16, isOutput=False)
    # combined constants [128, 5120]: cols [0:1024]=W stacked twice
    # (partitions 0-63 and 64-127 both hold W) | [1024:5120]=bias0..3
    cc_d = nc.declare_dram_parameter("cc", [TILE_P, 5 * D + TILE_P],
                                     mybir.dt.bfloat16, isOutput=False)
    out_d = nc.declare_dram_parameter("out", [ROWS, D], mybir.dt.bfloat16,
                                      isOutput=True)

    with tile.TileContext(nc) as tc:
        with (
            tc.tile_pool(name="const", bufs=1) as cpool,
            tc.tile_pool(name="outp", bufs=obufs) as opool,
            tc.tile_pool(name="ps", bufs=2, space="PSUM") as psp,
        ):
            # loads: sync ring carries A^T+x then the stores; scalar ring
            # carries W and the bias tiles in need-order.
            n0 = 9 if wide8 else 5 if wide0 else 2
            at_x0 = cpool.tile([TILE_P, n0 * TILE_P], mybir.dt.bfloat16)
            nc.sync.dma_start(at_x0[:], x_d[:, 0:n0 * TILE_P])
            at_ap = at_x0[:, 0:TILE_P]
            CCW = 5 * D + TILE_P
            BB = D + TILE_P            # bias block base in cc
            cc_sb = cpool.tile([TILE_P, CCW], mybir.dt.bfloat16)
            nc.scalar.dma_start(cc_sb[:, 0:BB], cc_d[:, 0:BB])
            xr_sb = cpool.tile([TILE_P, (N_PAIR - 1) * TILE_P],
                               mybir.dt.bfloat16)
            if wide8:
                pass
            elif wide0:
                nc.sync.dma_start(xr_sb[:, 3 * TILE_P:7 * TILE_P],
                                  x_d[:, 5 * TILE_P:9 * TILE_P])
            elif xsplit:
                nc.sync.dma_start(xr_sb[:, 0:3 * TILE_P],
                                  x_d[:, 2 * TILE_P:5 * TILE_P])
                nc.sync.dma_start(xr_sb[:, 3 * TILE_P:7 * TILE_P],
                                  x_d[:, 5 * TILE_P:9 * TILE_P])
            else:
                nc.sync.dma_start(xr_sb[:, 0:7 * TILE_P],
                                  x_d[:, 2 * TILE_P:9 * TILE_P])
            nc.sync.dma_start(xr_sb[:, 7 * TILE_P:15 * TILE_P],
                              x_d[:, 9 * TILE_P:17 * TILE_P])
            nc.scalar.dma_start(cc_sb[:, BB:BB + 2 * D],
                                cc_d[:, BB:BB + 2 * D])
            nc.scalar.dma_start(cc_sb[:, BB + 2 * D:CCW],
                                cc_d[:, BB + 2 * D:CCW])
            i_ap = cc_sb[:, D:D + TILE_P]

            def bias_pair(j):
                base = BB + ((2 * j) % N_BIAS) * D
                return cc_sb[:, base:base + PW]

            def w_ap(u, lo, hi):
                return cc_sb[64 * u:64 * u + F, lo:hi]

            # resident x~^T (bf16): xt_sb[64u+f, 128j+n] = x~[2j+u, n, f]
            xt_sb = cpool.tile([TILE_P, N_PAIR * TILE_P], mybir.dt.bfloat16)

            # prologue: the 16 transpose+mix matmuls use banks 0-1 of
            # pair 0's and pair 1's psum tiles as workspace (so each
            # pair's m3/m4 banks carry no WAR dependency on the late
            # transpose blocks), interleaved with the xt_sb copies.
            pair0 = psp.tile([TILE_P, PW], mybir.dt.float32,
                             name="pair", tag="pair")
            pair1 = psp.tile([TILE_P, PW], mybir.dt.float32,
                             name="pair", tag="pair")

            def tr_block(b4):
                # 4 transpose+mix matmuls into workspace banks, then one
                # bulk copy to xt_sb (ACT for even blocks, DVE for odd)
                wtile = pair0 if b4 < 2 else pair1
                wcol = 512 * (b4 % 2)
                for k in range(4):
                    pr = 4 * b4 + k
                    if wide8 and pr < 8:
                        src = at_x0[:, TILE_P * (pr + 1):TILE_P * (pr + 2)]
                    elif wide0 and pr < 4:
                        src = at_x0[:, TILE_P * (pr + 1):TILE_P * (pr + 2)]
                    elif pr == 0:
                        src = at_x0[:, TILE_P:2 * TILE_P]
                    else:
                        src = xr_sb[:, 128 * (pr - 1):128 * pr]
                    nc.tensor.matmul(wtile[:, wcol + 128 * k:
                                           wcol + 128 * (k + 1)],
                                     src, at_ap, start=True, stop=True)
                if b4 % 2 == 0:
                    nc.scalar.copy(xt_sb[:, 512 * b4:512 * (b4 + 1)],
                                   wtile[:, wcol:wcol + 512])
                else:
                    nc.vector.tensor_copy(xt_sb[:, 512 * b4:512 * (b4 + 1)],
                                          wtile[:, wcol:wcol + 512])

            state = {"prev": None}

            def pair_body(j):
                bias = bias_pair(j)
                pair = (pair0 if j == 0 else pair1 if j == 1 else psp.tile(
                    [TILE_P, PW], mybir.dt.float32, name="pair", tag="pair"))
                lhs0 = xt_sb[0:64, 128 * j:128 * (j + 1)]
                lhs1 = xt_sb[64:128, 128 * j:128 * (j + 1)]
                if mm1024:
                    nc.tensor.matmul(pair[:, 0:1024], lhs0,
                                     w_ap(0, 0, 1024), start=True, stop=True)
                    nc.tensor.matmul(pair[:, 1024:2048], lhs1,
                                     w_ap(1, 0, 1024), start=True, stop=True)
                else:
                    nc.tensor.matmul(pair[:, 0:512], lhs0,
                                     w_ap(0, 0, 512), start=True, stop=True)
                    nc.tensor.matmul(pair[:, 512:1024], lhs0,
                                     w_ap(0, 512, 1024),
                                     start=True, stop=True)
                    nc.tensor.matmul(pair[:, 1024:1536], lhs1,
                                     w_ap(1, 0, 512), start=True, stop=True)
                    nc.tensor.matmul(pair[:, 1536:2048], lhs1,
                                     w_ap(1, 512, 1024),
                                     start=True, stop=True)
                if inject:
                    nc.tensor.matmul(pair[:, 1536:2048], i_ap,
                                     bias[:, 1536:2048], start=False,
                                     stop=True, skip_group_check=True)
                o_t = opool.tile([TILE_P, PW], mybir.dt.bfloat16,
                                 name="o_t")
                if tailfuse and j == N_PAIR - 1:
                    # final pair: single fused DVE op, stores on the idle
                    # scalar ring -> shortest possible tail chain
                    nc.vector.tensor_add(o_t[:], pair[:], bias[:])
                    nc.scalar.dma_start(out_d[256 * j:256 * j + 128, :],
                                        o_t[:, 0:D])
                    nc.scalar.dma_start(
                        out_d[256 * j + 128:256 * j + 256, :],
                        o_t[:, D:PW])
                    if state["prev"] is not None:
                        _finish_pair(nc, out_d, *state["prev"], cfg)
                    state["prev"] = None
                    return
                # DVE fused drain+bias (PSUM 1x), ACT plain drain (1x)
                nc.vector.tensor_add(o_t[:, 0:xv], pair[:, 0:xv],
                                     bias[:, 0:xv])
                nc.scalar.copy(o_t[:, xv:PW], pair[:, xv:PW])
                if state["prev"] is not None:
                    _finish_pair(nc, out_d, *state["prev"], cfg)
                state["prev"] = (j, o_t, bias)

            # ramp: pair 0 runs between transpose blocks so its mains are
            # not queued behind the full transpose sweep on PE/ACT
            tr_block(0)
            tr_block(1)
            pair_body(0)
            if late23:
                with tc.tile_wait_until(0.004):
                    tr_block(2)
                    tr_block(3)
            else:
                tr_block(2)
                tr_block(3)
            for j in range(1, N_PAIR):
                if steadyhint:
                    with tc.tile_wait_until(0.004 + 0.0017 * j):
                        pair_body(j)
                else:
                    pair_body(j)
            if state["prev"] is not None:
                _finish_pair(nc, out_d, *state["prev"], cfg)
    nc.compile()
    return nc


def _finish_pair(nc, out_d, j, o_t, bias, cfg):
    # bias add for ACT's drained region: bf16 SBUF tensor_tensor (2x),
    # in place; then both stores.
    xv = cfg["XV"]
    hi = 1536 if cfg["INJECT"] else PW
    gps = cfg["GPS"]
    if gps:
        nc.vector.tensor_add(o_t[:, xv:hi - gps], o_t[:, xv:hi - gps],
                             bias[:, xv:hi - gps])
        nc.gpsimd.tensor_add(o_t[:, hi - gps:hi], o_t[:, hi - gps:hi],
                             bias[:, hi - gps:hi])
    else:
        nc.vector.tensor_add(o_t[:, xv:hi], o_t[:, xv:hi], bias[:, xv:hi])
    if cfg["TAILSPLIT"] and j == N_PAIR - 1:
        for q in range(4):
            r0 = 256 * j + 128 * (q // 2)
            c0 = 512 * (q % 2)
            nc.sync.dma_start(out_d[r0:r0 + 128, c0:c0 + 512],
                              o_t[:, 512 * q:512 * (q + 1)])
        return
    nc.sync.dma_start(out_d[256 * j:256 * j + 128, :], o_t[:, 0:D])
    nc.sync.dma_start(out_d[256 * j + 128:256 * j + 256, :], o_t[:, D:PW])


def _host_constants(W_emb, b_emb, w_seg, b_seg):
    # sinusoidal positional encoding, float32, same formula as the reference
    pos = np.arange(S, dtype=np.float32)[:, None]
    div = np.exp(np.arange(0, D, 2, dtype=np.float32)
                 * (-np.log(10000.0) / D)).astype(np.float32)
    ang = pos * div
    pe = np.zeros((S, D), np.float32)
    pe[:, 0::2] = np.sin(ang)
    pe[:, 1::2] = np.cos(ang)

    bias = (pe + b_emb[None, :] * (np.float32(1.0) + w_seg.sum())
            + b_seg[0]).astype(np.float32)
    # rearrange to [128, 4*D]: column block j holds bias rows j*128..j*128+127
    bias_r = np.ascontiguousarray(
        bias.reshape(N_BIAS, TILE_P, D).transpose(1, 0, 2).reshape(
            TILE_P, N_BIAS * D)).astype(ml_dtypes.bfloat16)

    blk = np.eye(SEG, dtype=np.float32) + w_seg[:, None] * np.ones(
        (1, SEG), np.float32)
    at = np.kron(np.eye(TILE_P // SEG, dtype=np.float32), blk).astype(
        ml_dtypes.bfloat16)

    wb = np.vstack([W_emb, W_emb]).astype(ml_dtypes.bfloat16)
    ident = np.eye(TILE_P, dtype=np.float32).astype(ml_dtypes.bfloat16)
    # combined consts: [W2|I128|bias0..3] as [128, 5*D+128] bf16
    cc = np.ascontiguousarray(np.concatenate([wb, ident, bias_r], axis=1))
    return at, cc


def _prepare_in_maps(x, W_emb, b_emb, w_seg, b_seg):
    x = np.ascontiguousarray(np.asarray(x, dtype=np.float32))
    W_emb = np.asarray(W_emb, dtype=np.float32)
    b_emb = np.asarray(b_emb, dtype=np.float32)
    w_seg = np.asarray(w_seg, dtype=np.float32)
    b_seg = np.asarray(b_seg, dtype=np.float32)

    at, cc = _host_constants(W_emb, b_emb, w_seg, b_seg)

    in_maps = []
    for c in range(N_CORES):
        xs = x[c * B_LOC:(c + 1) * B_LOC].reshape(ROWS, F)
        # rearrange [32 tiles, 128 rows, F] -> [128, 32*F], bf16 staging
        xr = np.ascontiguousarray(
            xs.reshape(N_TILES, TILE_P, F).transpose(1, 0, 2).reshape(
                TILE_P, N_TILES * F)).astype(ml_dtypes.bfloat16)
        in_maps.append(
            {"x": np.ascontiguousarray(np.concatenate([at, xr], axis=1)),
             "cc": cc})
    return in_maps


def kernel(x, W_emb, b_emb, w_seg, b_seg):
    in_maps = _prepare_in_maps(x, W_emb, b_emb, w_seg, b_seg)

    global _NC_CACHE
    if _NC_CACHE is None:
        _NC_CACHE = _build_nc()

    res = run_bass_kernel_spmd(_NC_CACHE, in_maps,
                               core_ids=list(range(N_CORES)))
    out = np.concatenate(
        [np.asarray(res.results[c]["out"]).astype(np.float32).reshape(
            B_LOC, S, D) for c in range(N_CORES)], axis=0)
    return out


# revision 24
# speedup vs baseline: 1.0664x; 1.0094x over previous
"""BERT input representation kernel for 8 TRN2 NeuronCores.

Math (reference):
    x1  = x @ W_emb + b_emb                      # [B,S,D]
    seg = einsum('bnsd,s->bnd', x1.reshape(B,S/8,8,D), w_seg) + b_seg
    out = (x1.reshape(...) + seg[:,:,None,:]).reshape(B,S,D) + PE(S,D)

Folded form used here (exact algebra):
    out[b,s,:] = (A @ x[b])[s,:] @ W_emb + bias[s,:]
where A = I + blockdiag(ones(8,1) @ w_seg[None,:]) mixes rows within each
8-row segment, and bias[s,:] = PE[s,:] + b_emb*(1 + sum(w_seg)) + b_seg.

Sharding: pure data-parallel over batch; each of 8 cores handles 8
batches (4096 rows = 32 row-tiles of 128 rows = 16 tile-pair groups).

Schedule (v6):
  - output stored bf16 (host upcasts to f32): store traffic halves vs
    f32, 8.4 MiB/core written, ~2 MiB read
  - prologue: the first sync-ring DMA carries A^T + x for pairs 0-7
    (288 KB; its completion receipt gates the whole ramp, and is only
    ~0.4 us later than a minimal load), a second carries pairs 8-15;
    W+identity then bias tiles go on the scalar ring; the 16 transpose+segment-mix matmuls use banks
    0-1 of pair 0/1's PSUM tiles as workspace, with bulk PSUM->SBUF
    copies (alternating ACT/DVE) building the resident bf16 x~^T.
    Pair 0's body is emitted between transpose blocks 1 and 2, and
    blocks 2-3 carry a late scheduling hint (tile_wait_until), so the
    first store launches as early as possible.
  - steady loop per pair j: one [128,2048] f32 PSUM tile (4 banks,
    2 bufs = all of PSUM), 4 mains (start=True, FD=512).  Epilogue is
    split across both PSUM-capable engines: DVE does a fused
    drain+bias tensor_tensor on cols [0:XV) (PSUM 1x mode), ACT
    plain-drains [XV:2048) (1x), and DVE adds bias there as a bf16
    SBUF tensor_tensor (2x packed mode).  The DVE add for pair j is
    emitted after pair j+1's fused op (software pipelining) so DVE
    never idles waiting for ACT.  Steady cadence ~1.7 us/pair,
    ACT/DVE-balanced (the f32 PSUM drain at 1 elem/cycle/lane is the
    hard floor on TRN2; matmul cannot write bf16 PSUM here).
  - two 256 KiB bf16 stores per pair on the sync HWDGE ring; the
    final pair's epilogue runs as one fused DVE op with its stores on
    the otherwise-idle scalar ring (shortest tail chain before the
    last store's HBM write receipt, which gates the NEFF end barrier)
"""

import sys

if "/opt/trn_rl_repo" not in sys.path:
    sys.path.insert(0, "/opt/trn_rl_repo")

import ml_dtypes
import numpy as np

import concourse.bacc as bacc
import concourse.mybir as mybir
import concourse.tile as tile
from concourse.bass_utils import run_bass_kernel_spmd

B, S, F, D, SEG = 64, 512, 64, 1024, 8
N_CORES = 8
B_LOC = B // N_CORES          # batches per core
ROWS = B_LOC * S              # 4096 rows per core
TILE_P = 128                  # rows per tile
N_TILES = ROWS // TILE_P      # 32
N_PAIR = N_TILES // 2         # 16 tile-pairs
N_BIAS = S // TILE_P          # 4 distinct bias row-tiles
PW = 2 * D                    # 2048 cols per pair psum tile
XV = 576                      # DVE fused drain+bias covers cols [0:XV)

_NC_CACHE = None
DEFAULT_CFG = {"XV": XV, "INJECT": False, "LATE23": True, "XSPLIT": False,
               "WIDE0": False, "TAILSPLIT": False, "OBUFS": 4, "GPS": 0,
               "MM1024": False, "TAILFUSE": True, "STEADYHINT": False,
               "WIDE8": True, "FLIP01": False}


def _build_nc(cfg=None):
    cfg = dict(DEFAULT_CFG, **(cfg or {}))
    xv, inject = cfg["XV"], cfg["INJECT"]
    late23, xsplit = cfg["LATE23"], cfg["XSPLIT"]
    wide0, tailsplit = cfg["WIDE0"], cfg["TAILSPLIT"]
    obufs = cfg["OBUFS"]
    mm1024 = cfg["MM1024"]
    tailfuse = cfg["TAILFUSE"]
    steadyhint = cfg["STEADYHINT"]
    wide8 = cfg["WIDE8"]
    flip01 = cfg["FLIP01"]
    nc = bacc.Bacc("TRN2", target_bir_lowering=False, debug=False,
                   num_devices=N_CORES)
    # x pre-rearranged on host (layout + cast to bf16):
    # xr[p, i*F:(i+1)*F] = x[i*128+p]; cols [0:128] = A^T
    x_d = nc.declare_dram_parameter("x", [TILE_P, TILE_P + N_TILES * F],
                                    mybir.dt.bfloat16, isOutput=False)
    # combined constants [128, 5120]: cols [0:1024]=W stacked twice
    # (partitions 0-63 and 64-127 both hold W) | [1024:5120]=bias0..3
    cc_d = nc.declare_dram_parameter("cc", [TILE_P, 5 * D + TILE_P],
                                     mybir.dt.bfloat16, isOutput=False)
    out_d = nc.declare_dram_parameter("out", [ROWS, D], mybir.dt.bfloat16,
                                      isOutput=True)

    with tile.TileContext(nc) as tc:
        with (
            tc.tile_pool(name="const", bufs=1) as cpool,
            tc.tile_pool(name="outp", bufs=obufs) as opool,
            tc.tile_pool(name="ps", bufs=2, space="PSUM") as psp,
        ):
            # loads: sync ring carries A^T+x then the stores; scalar ring
            # carries W and the bias tiles in need-order.
            n0 = 9 if wide8 else 5 if wide0 else 2
            at_x0 = cpool.tile([TILE_P, n0 * TILE_P], mybir.dt.bfloat16)
            nc.sync.dma_start(at_x0[:], x_d[:, 0:n0 * TILE_P])
            at_ap = at_x0[:, 0:TILE_P]
            CCW = 5 * D + TILE_P
            BB = D + TILE_P            # bias block base in cc
            cc_sb = cpool.tile([TILE_P, CCW], mybir.dt.bfloat16)
            nc.scalar.dma_start(cc_sb[:, 0:BB], cc_d[:, 0:BB])
            xr_sb = cpool.tile([TILE_P, (N_PAIR - 1) * TILE_P],
                               mybir.dt.bfloat16)
            if wide8:
                pass
            elif wide0:
                nc.sync.dma_start(xr_sb[:, 3 * TILE_P:7 * TILE_P],
                                  x_d[:, 5 * TILE_P:9 * TILE_P])
            elif xsplit:
                nc.sync.dma_start(xr_sb[:, 0:3 * TILE_P],
                                  x_d[:, 2 * TILE_P:5 * TILE_P])
                nc.sync.dma_start(xr_sb[:, 3 * TILE_P:7 * TILE_P],
                                  x_d[:, 5 * TILE_P:9 * TILE_P])
            else:
                nc.sync.dma_start(xr_sb[:, 0:7 * TILE_P],
                                  x_d[:, 2 * TILE_P:9 * TILE_P])
            nc.sync.dma_start(xr_sb[:, 7 * TILE_P:15 * TILE_P],
                              x_d[:, 9 * TILE_P:17 * TILE_P])
            nc.scalar.dma_start(cc_sb[:, BB:BB + 2 * D],
                                cc_d[:, BB:BB + 2 * D])
            nc.scalar.dma_start(cc_sb[:, BB + 2 * D:CCW],
                                cc_d[:, BB + 2 * D:CCW])
            i_ap = cc_sb[:, D:D + TILE_P]

            def bias_pair(j):
                base = BB + ((2 * j) % N_BIAS) * D
                return cc_sb[:, base:base + PW]

            def w_ap(u, lo, hi):
                return cc_sb[64 * u:64 * u + F, lo:hi]

            # resident x~^T (bf16): xt_sb[64u+f, 128j+n] = x~[2j+u, n, f]
            xt_sb = cpool.tile([TILE_P, N_PAIR * TILE_P], mybir.dt.bfloat16)

            # prologue: the 16 transpose+mix matmuls use banks 0-1 of
            # pair 0's and pair 1's psum tiles as workspace (so each
            # pair's m3/m4 banks carry no WAR dependency on the late
            # transpose blocks), interleaved with the xt_sb copies.
            pair0 = psp.tile([TILE_P, PW], mybir.dt.float32,
                             name="pair", tag="pair")
            pair1 = psp.tile([TILE_P, PW], mybir.dt.float32,
                             name="pair", tag="pair")

            def tr_block(b4):
                # 4 transpose+mix matmuls into workspace banks, then one
                # bulk copy to xt_sb (ACT for even blocks, DVE for odd)
                wtile = pair0 if b4 < 2 else pair1
                wcol = 512 * (b4 % 2)
                for k in range(4):
                    pr = 4 * b4 + k
                    if wide8 and pr < 8:
                        src = at_x0[:, TILE_P * (pr + 1):TILE_P * (pr + 2)]
                    elif wide0 and pr < 4:
                        src = at_x0[:, TILE_P * (pr + 1):TILE_P * (pr + 2)]
                    elif pr == 0:
                        src = at_x0[:, TILE_P:2 * TILE_P]
                    else:
                        src = xr_sb[:, 128 * (pr - 1):128 * pr]
                    nc.tensor.matmul(wtile[:, wcol + 128 * k:
                                           wcol + 128 * (k + 1)],
                                     src, at_ap, start=True, stop=True)
                if b4 % 2 == 0:
                    nc.scalar.copy(xt_sb[:, 512 * b4:512 * (b4 + 1)],
                                   wtile[:, wcol:wcol + 512])
                else:
                    nc.vector.tensor_copy(xt_sb[:, 512 * b4:512 * (b4 + 1)],
                                          wtile[:, wcol:wcol + 512])

            state = {"prev": None}

            def pair_body(j):
                bias = bias_pair(j)
                pair = (pair0 if j == 0 else pair1 if j == 1 else psp.tile(
                    [TILE_P, PW], mybir.dt.float32, name="pair", tag="pair"))
                lhs0 = xt_sb[0:64, 128 * j:128 * (j + 1)]
                lhs1 = xt_sb[64:128, 128 * j:128 * (j + 1)]
                if flip01 and j < 2:
                    # ramp pairs: banks 0-1 are the transpose workspace,
                    # so run the free banks 2-3 first and flip the
                    # epilogue split: DVE fuses [1024:2048), ACT drains
                    # [0:1024) once the workspace copies release it.
                    nc.tensor.matmul(pair[:, 1024:1536], lhs1,
                                     w_ap(1, 0, 512), start=True, stop=True)
                    nc.tensor.matmul(pair[:, 1536:2048], lhs1,
                                     w_ap(1, 512, 1024),
                                     start=True, stop=True)
                    nc.tensor.matmul(pair[:, 0:512], lhs0,
                                     w_ap(0, 0, 512), start=True, stop=True)
                    nc.tensor.matmul(pair[:, 512:1024], lhs0,
                                     w_ap(0, 512, 1024),
                                     start=True, stop=True)
                    o_t = opool.tile([TILE_P, PW], mybir.dt.bfloat16,
                                     name="o_t")
                    nc.vector.tensor_add(o_t[:, D:PW], pair[:, D:PW],
                                         bias[:, D:PW])
                    nc.scalar.copy(o_t[:, 0:D], pair[:, 0:D])
                    if state["prev"] is not None:
                        _finish_pair(nc, out_d, *state["prev"], cfg)
                    state["prev"] = (j, o_t, bias, 0, D)
                    return
                if mm1024:
                    nc.tensor.matmul(pair[:, 0:1024], lhs0,
                                     w_ap(0, 0, 1024), start=True, stop=True)
                    nc.tensor.matmul(pair[:, 1024:2048], lhs1,
                                     w_ap(1, 0, 1024), start=True, stop=True)
                else:
                    nc.tensor.matmul(pair[:, 0:512], lhs0,
                                     w_ap(0, 0, 512), start=True, stop=True)
                    nc.tensor.matmul(pair[:, 512:1024], lhs0,
                                     w_ap(0, 512, 1024),
                                     start=True, stop=True)
                    nc.tensor.matmul(pair[:, 1024:1536], lhs1,
                                     w_ap(1, 0, 512), start=True, stop=True)
                    nc.tensor.matmul(pair[:, 1536:2048], lhs1,
                                     w_ap(1, 512, 1024),
                                     start=True, stop=True)
                if inject:
                    nc.tensor.matmul(pair[:, 1536:2048], i_ap,
                                     bias[:, 1536:2048], start=False,
                                     stop=True, skip_group_check=True)
                o_t = opool.tile([TILE_P, PW], mybir.dt.bfloat16,
                                 name="o_t")
                if tailfuse and j == N_PAIR - 1:
                    # final pair: single fused DVE op, stores on the idle
                    # scalar ring -> shortest possible tail chain
                    nc.vector.tensor_add(o_t[:], pair[:], bias[:])
                    nc.scalar.dma_start(out_d[256 * j:256 * j + 128, :],
                                        o_t[:, 0:D])
                    nc.scalar.dma_start(
                        out_d[256 * j + 128:256 * j + 256, :],
                        o_t[:, D:PW])
                    if state["prev"] is not None:
                        _finish_pair(nc, out_d, *state["prev"], cfg)
                    state["prev"] = None
                    return
                # DVE fused drain+bias (PSUM 1x), ACT plain drain (1x)
                nc.vector.tensor_add(o_t[:, 0:xv], pair[:, 0:xv],
                                     bias[:, 0:xv])
                nc.scalar.copy(o_t[:, xv:PW], pair[:, xv:PW])
                if state["prev"] is not None:
                    _finish_pair(nc, out_d, *state["prev"], cfg)
                state["prev"] = (j, o_t, bias, xv,
                                 1536 if inject else PW)

            # ramp: pair 0 runs between transpose blocks so its mains are
            # not queued behind the full transpose sweep on PE/ACT
            tr_block(0)
            tr_block(1)
            pair_body(0)
            if late23:
                with tc.tile_wait_until(0.004):
                    tr_block(2)
                    tr_block(3)
            else:
                tr_block(2)
                tr_block(3)
            for j in range(1, N_PAIR):
                if steadyhint:
                    with tc.tile_wait_until(0.004 + 0.0017 * j):
                        pair_body(j)
                else:
                    pair_body(j)
            if state["prev"] is not None:
                _finish_pair(nc, out_d, *state["prev"], cfg)
    nc.compile()
    return nc


def _finish_pair(nc, out_d, j, o_t, bias, xv, hi, cfg):
    # bias add for ACT's drained region: bf16 SBUF tensor_tensor (2x),
    # in place; then both stores.
    gps = cfg["GPS"]
    if gps:
        nc.vector.tensor_add(o_t[:, xv:hi - gps], o_t[:, xv:hi - gps],
                             bias[:, xv:hi - gps])
        nc.gpsimd.tensor_add(o_t[:, hi - gps:hi], o_t[:, hi - gps:hi],
                             bias[:, hi - gps:hi])
    else:
        nc.vector.tensor_add(o_t[:, xv:hi], o_t[:, xv:hi], bias[:, xv:hi])
    if cfg["TAILSPLIT"] and j == N_PAIR - 1:
        for q in range(4):
            r0 = 256 * j + 128 * (q // 2)
            c0 = 512 * (q % 2)
            nc.sync.dma_start(out_d[r0:r0 + 128, c0:c0 + 512],
                              o_t[:, 512 * q:512 * (q + 1)])
        return
    nc.sync.dma_start(out_d[256 * j:256 * j + 128, :], o_t[:, 0:D])
    nc.sync.dma_start(out_d[256 * j + 128:256 * j + 256, :], o_t[:, D:PW])


def _host_constants(W_emb, b_emb, w_seg, b_seg):
    # sinusoidal positional encoding, float32, same formula as the reference
    pos = np.arange(S, dtype=np.float32)[:, None]
    div = np.exp(np.arange(0, D, 2, dtype=np.float32)
                 * (-np.log(10000.0) / D)).astype(np.float32)
    ang = pos * div
    pe = np.zeros((S, D), np.float32)
    pe[:, 0::2] = np.sin(ang)
    pe[:, 1::2] = np.cos(ang)

    bias = (pe + b_emb[None, :] * (np.float32(1.0) + w_seg.sum())
            + b_seg[0]).astype(np.float32)
    # rearrange to [128, 4*D]: column block j holds bias rows j*128..j*128+127
    bias_r = np.ascontiguousarray(
        bias.reshape(N_BIAS, TILE_P, D).transpose(1, 0, 2).reshape(
            TILE_P, N_BIAS * D)).astype(ml_dtypes.bfloat16)

    blk = np.eye(SEG, dtype=np.float32) + w_seg[:, None] * np.ones(
        (1, SEG), np.float32)
    at = np.kron(np.eye(TILE_P // SEG, dtype=np.float32), blk).astype(
        ml_dtypes.bfloat16)

    wb = np.vstack([W_emb, W_emb]).astype(ml_dtypes.bfloat16)
    ident = np.eye(TILE_P, dtype=np.float32).astype(ml_dtypes.bfloat16)
    # combined consts: [W2|I128|bias0..3] as [128, 5*D+128] bf16
    cc = np.ascontiguousarray(np.concatenate([wb, ident, bias_r], axis=1))
    return at, cc


def _prepare_in_maps(x, W_emb, b_emb, w_seg, b_seg):
    x = np.ascontiguousarray(np.asarray(x, dtype=np.float32))
    W_emb = np.asarray(W_emb, dtype=np.float32)
    b_emb = np.asarray(b_emb, dtype=np.float32)
    w_seg = np.asarray(w_seg, dtype=np.float32)
    b_seg = np.asarray(b_seg, dtype=np.float32)

    at, cc = _host_constants(W_emb, b_emb, w_seg, b_seg)

    in_maps = []
    for c in range(N_CORES):
        xs = x[c * B_LOC:(c + 1) * B_LOC].reshape(ROWS, F)
        # rearrange [32 tiles, 128 rows, F] -> [128, 32*F], bf16 staging
        xr = np.ascontiguousarray(
            xs.reshape(N_TILES, TILE_P, F).transpose(1, 0, 2).reshape(
                TILE_P, N_TILES * F)).astype(ml_dtypes.bfloat16)
        in_maps.append(
            {"x": np.ascontiguousarray(np.concatenate([at, xr], axis=1)),
             "cc": cc})
    return in_maps


def kernel(x, W_emb, b_emb, w_seg, b_seg):
    in_maps = _prepare_in_maps(x, W_emb, b_emb, w_seg, b_seg)

    global _NC_CACHE
    if _NC_CACHE is None:
        _NC_CACHE = _build_nc()

    res = run_bass_kernel_spmd(_NC_CACHE, in_maps,
                               core_ids=list(range(N_CORES)))
    out = np.concatenate(
        [np.asarray(res.results[c]["out"]).astype(np.float32).reshape(
            B_LOC, S, D) for c in range(N_CORES)], axis=0)
    return out
